# revision 20
# baseline (speedup 1.0000x reference)
"""Bass/Tile attention kernel for Trainium2, SPMD over 8 NeuronCores.

Problem: B,S,D,DK = 8,2048,512,64 full bidirectional attention with
softmax; returns (attended[B,S,DK], weights[B,S,S]).

Sharding: data-parallel over batch — core b handles batch b. No
collectives needed. W_q/W_k/W_v replicated.

Per-core dataflow (S=2048 seq, D=512 model, DK=64 head), bf16 compute:
  x --SWDGE cast-DMA--> xb bf16 --DMA xbar transpose--> xT[512,2048]
  qT/kT[64,2048] = W.T @ xT (zero-padded to 128 partitions: K=128
  matmuls are ~2x faster than K=64) ; vT --PE transpose--> v[2048,64]
  main loop over 16 query tiles t (software-pipelined depth 2):
    S_t[128,2048] = qT_t.T @ kT            (PE, 2 PSUM halves)
    expS_t = exp(0.125*S_t)                (ACT, accum_out -> Z_t)
    P_t = expS_t * (1/Z_t)  (GpSimd, bf16) --> DMA out (bf16 wts)
    PT_t = PE-transpose expS_t (16 tiles)  --> ptcols_t (DVE copies)
    att_t[128,64] = sum_u PT_t[u].T @ v_u  (PE, per-tile PSUM chain)
    att_t *= 1/Z_t --> DMA out (f32)
"""

import numpy as np

B, S, D, DK = 8, 2048, 512, 64
P = 128          # partition size
SQT = S // P     # 16 query tiles
C = D // P       # 4 d-chunks
U = S // P       # 16 key tiles
NCH = S // 512   # 4 free-dim 512-chunks

_CACHE = {}


def _build():
    from concourse import bacc, mybir, tile
    from concourse.masks import make_identity

    f32 = mybir.dt.float32
    bf16 = mybir.dt.bfloat16
    i32 = mybir.dt.int32
    Exp = mybir.ActivationFunctionType.Exp

    nc = bacc.Bacc("TRN2", target_bir_lowering=False, debug=False,
                   num_devices=B)

    x_d = nc.dram_tensor("x", [S, D], f32, kind="ExternalInput").ap()
    wq_d = nc.dram_tensor("wq", [D, DK], f32, kind="ExternalInput").ap()
    wk_d = nc.dram_tensor("wk", [D, DK], f32, kind="ExternalInput").ap()
    wv_d = nc.dram_tensor("wv", [D, DK], f32, kind="ExternalInput").ap()
    att_d = nc.dram_tensor("att", [S, DK], f32, kind="ExternalOutput").ap()
    wts_d = nc.dram_tensor("wts", [S, S], bf16, kind="ExternalOutput").ap()

    with tile.TileContext(nc) as tc:
        with tc.tile_pool(name="const", bufs=1) as const:
            ident_b = const.tile([P, P], bf16)
            make_identity(nc, ident_b)

            # xT4[p, t, c, j] = x[t*128+j, c*128+p] as bf16
            xT4 = const.tile([P, SQT, C, P], bf16)
            # qT/kT zero-padded from DK=64 to 128 partitions
            qT = const.tile([P, S], bf16)
            kT = const.tile([P, S], bf16)
            vT = const.tile([DK, S], bf16)
            v_sb = const.tile([P, U * DK], bf16)
            recip = const.tile([P, SQT], f32)   # 1/Z per query tile
            nc.gpsimd.memset(qT[DK:, :], 0.0)
            nc.gpsimd.memset(kT[DK:, :], 0.0)

            # ---------------- setup phase (own PSUM pools) ----------------
            with (
                tc.tile_pool(name="xstage", bufs=1) as xstage,
                tc.tile_pool(name="trps", bufs=2, space="PSUM") as trps,
                tc.tile_pool(name="qkvps", bufs=2, space="PSUM") as qkvps,
            ):
                # weights: [512,64] f32 -> [128, 3, c, 64] bf16
                wstage = xstage.tile([P, 3, C, DK], f32, tag="wstage")
                for i, w_d in enumerate((wq_d, wk_d, wv_d)):
                    nc.sync.dma_start(
                        wstage[:, i], w_d.rearrange("(c p) k -> p c k", p=P))
                w_sb3 = const.tile([P, 3, C, DK], bf16)
                nc.vector.tensor_copy(w_sb3[:], wstage[:])

                # x: SWDGE DMA with fused f32->bf16 cast, in 4 chunks,
                # each followed by a DMA xbar transpose into xT4.
                xb = xstage.tile([P, SQT, D], bf16, tag="xb")
                for q in range(4):
                    csl = slice(4 * q, 4 * (q + 1))
                    nc.gpsimd.dma_start(
                        xb[:, csl],
                        x_d.rearrange("(t p) d -> p t d", p=P)[:, csl])
                    nc.sync.dma_start_transpose(
                        xT4[:, csl], xb[:, csl].rearrange("p t d -> p (t d)"))

                # qT, kT, vT = W.T @ xT
                for n in range(NCH):
                    for w_i, dst in ((0, qT), (1, kT), (2, vT)):
                        mm = qkvps.tile([DK, 512], f32, tag="qkv")
                        for c in range(C):
                            nc.tensor.matmul(
                                mm[:], w_sb3[:, w_i, c, :],
                                xT4[:, 4 * n:4 * (n + 1), c, :],
                                start=(c == 0), stop=(c == C - 1))
                        nc.vector.tensor_copy(
                            dst[:DK, n * 512:(n + 1) * 512], mm[:])

                # v natural [128, u*64] via PE transpose of vT
                for g in range(2):
                    trp = trps.tile([P, 512], bf16, tag="tr")
                    for j in range(8):
                        u = g * 8 + j
                        nc.tensor.transpose(
                            trp[:, j * DK:(j + 1) * DK],
                            vT[:, u * P:(u + 1) * P], ident_b[:DK, :DK])
                    nc.vector.tensor_copy(
                        v_sb[:, g * 512:(g + 1) * 512], trp[:])

            # ---------------- main loop (own PSUM pools) ----------------
            with (
                tc.tile_pool(name="expp", bufs=2) as expp,
                tc.tile_pool(name="ptc", bufs=2) as ptc,
                tc.tile_pool(name="pout", bufs=3) as pout,
                tc.tile_pool(name="spsum", bufs=2, space="PSUM") as spsum,
                tc.tile_pool(name="trpsum", bufs=2, space="PSUM") as trpsum,
                tc.tile_pool(name="attpsum", bufs=2, space="PSUM") as attps,
            ):
                expS_hist = {}
                ptcols_hist = {}

                def stage_s(t):
                    # S_t matmuls + exp + normalize + weights DMA out
                    tsl = slice(t * P, (t + 1) * P)
                    expS = expp.tile([P, S], bf16, tag="exp")
                    zp = expp.tile([P, 2], f32, tag="zp")
                    for h in range(2):
                        sp = spsum.tile([P, 1024], f32, tag="s")
                        for i in range(2):
                            n = h * 2 + i
                            nc.tensor.matmul(
                                sp[:, i * 512:(i + 1) * 512],
                                qT[:, tsl], kT[:, n * 512:(n + 1) * 512],
                                start=True, stop=True)
                        nc.scalar.activation(
                            expS[:, h * 1024:(h + 1) * 1024], sp[:], Exp,
                            scale=0.125, accum_out=zp[:, h:h + 1])
                    zs = expp.tile([P, 1], f32, tag="zs")
                    nc.vector.tensor_add(zs[:], zp[:, 0:1], zp[:, 1:2])
                    nc.vector.reciprocal(recip[:, t:t + 1], zs[:])
                    p_t = pout.tile([P, S], bf16, tag="p")
                    nc.gpsimd.tensor_scalar_mul(
                        p_t[:], expS[:], recip[:, t:t + 1])
                    nc.sync.dma_start(wts_d[tsl, :], p_t[:])
                    expS_hist[t] = expS

                def stage_tr(t):
                    # PE-transpose expS_t into ptcols_t [128, u*128]
                    expS = expS_hist.pop(t)
                    ptcols = ptc.tile([P, U * P], bf16, tag="ptc")
                    for g in range(4):
                        trp = trpsum.tile([P, 512], bf16, tag="tr")
                        for j in range(4):
                            u = g * 4 + j
                            nc.tensor.transpose(
                                trp[:, j * P:(j + 1) * P],
                                expS[:, u * P:(u + 1) * P], ident_b[:])
                        nc.vector.tensor_copy(
                            ptcols[:, g * 512:(g + 1) * 512].bitcast(i32),
                            trp[:].bitcast(i32))
                    ptcols_hist[t] = ptcols

                def stage_pv(t):
                    # att_t = sum_u PT_t[u].T @ v_u ; scale; DMA out
                    ptcols = ptcols_hist.pop(t)
                    ap = attps.tile([P, DK], f32, tag="att")
                    for u in range(U):
                        nc.tensor.matmul(
                            ap[:], ptcols[:, u * P:(u + 1) * P],
                            v_sb[:, u * DK:(u + 1) * DK],
                            start=(u == 0), stop=(u == U - 1))
                    a_t = pout.tile([P, DK], f32, tag="a")
                    nc.vector.tensor_scalar_mul(
                        a_t[:], ap[:], recip[:, t:t + 1])
                    nc.sync.dma_start(att_d[t * P:(t + 1) * P, :], a_t[:])

                for t in range(SQT):
                    stage_s(t)
                    if t >= 1:
                        stage_tr(t - 1)
                    if t >= 2:
                        stage_pv(t - 2)
                stage_tr(SQT - 1)
                stage_pv(SQT - 2)
                stage_pv(SQT - 1)

    nc.compile()
    return nc


def get_nc():
    if "nc" not in _CACHE:
        _CACHE["nc"] = _build()
    return _CACHE["nc"]


def kernel(inputs, W_q, W_k, W_v):
    from concourse.bass_utils import run_bass_kernel_spmd

    nc = get_nc()
    inputs = np.ascontiguousarray(inputs, dtype=np.float32)
    in_maps = [
        {
            "x": inputs[b],
            "wq": np.ascontiguousarray(W_q, dtype=np.float32),
            "wk": np.ascontiguousarray(W_k, dtype=np.float32),
            "wv": np.ascontiguousarray(W_v, dtype=np.float32),
        }
        for b in range(B)
    ]
    res = run_bass_kernel_spmd(nc, in_maps, core_ids=list(range(B)))
    att = np.stack([res.results[b]["att"] for b in range(B)])
    wts = np.stack(
        [res.results[b]["wts"].astype(np.float32) for b in range(B)])
    return att, wts


# revision 23
# speedup vs baseline: 5.0658x; 5.0658x over previous
"""Bass/Tile attention kernel for Trainium2, SPMD over 8 NeuronCores.

Problem: B,S,D,DK = 8,2048,512,64 full bidirectional attention with
softmax; returns (attended[B,S,DK], weights[B,S,S]).

Sharding: data-parallel over batch — core b handles batch b. No
collectives needed. W_q/W_k/W_v replicated.

Per-core dataflow (S=2048 seq, D=512 model, DK=64 head), bf16 compute:
  x --SWDGE cast-DMA--> xb bf16 --DMA xbar transpose--> xT[512,2048]
  qT/kT[64,2048] = W.T @ xT (zero-padded to 128 partitions: K=128
  matmuls are ~2x faster than K=64) ; vT --PE transpose--> v[2048,64]
  main loop over 16 query tiles t (software-pipelined depth 2):
    S_t[128,2048] = qT_t.T @ kT            (PE, 2 PSUM halves)
    expS_t = exp(0.125*S_t)                (ACT, accum_out -> Z_t)
    P_t = expS_t * (1/Z_t)  (GpSimd, bf16) --> DMA out (bf16 wts)
    PT_t = PE-transpose expS_t (16 tiles)  --> ptcols_t (DVE copies)
    att_t[128,64] = sum_u PT_t[u].T @ v_u  (PE, per-tile PSUM chain)
    att_t *= 1/Z_t --> DMA out (f32)
"""

import numpy as np

B, S, D, DK = 8, 2048, 512, 64
P = 128          # partition size
SQT = S // P     # 16 query tiles
C = D // P       # 4 d-chunks
U = S // P       # 16 key tiles
NCH = S // 512   # 4 free-dim 512-chunks

_CACHE = {}


def _build():
    from concourse import bacc, mybir, tile
    from concourse.masks import make_identity

    f32 = mybir.dt.float32
    bf16 = mybir.dt.bfloat16
    i32 = mybir.dt.int32
    Exp = mybir.ActivationFunctionType.Exp

    nc = bacc.Bacc("TRN2", target_bir_lowering=False, debug=False,
                   num_devices=B)

    x_d = nc.dram_tensor("x", [S, D], f32, kind="ExternalInput").ap()
    wq_d = nc.dram_tensor("wq", [D, DK], f32, kind="ExternalInput").ap()
    wk_d = nc.dram_tensor("wk", [D, DK], f32, kind="ExternalInput").ap()
    wv_d = nc.dram_tensor("wv", [D, DK], f32, kind="ExternalInput").ap()
    att_d = nc.dram_tensor("att", [S, DK], f32, kind="ExternalOutput").ap()
    wts_d = nc.dram_tensor("wts", [S, S], bf16, kind="ExternalOutput").ap()

    with tile.TileContext(nc) as tc:
        with tc.tile_pool(name="const", bufs=1) as const:
            ident_b = const.tile([P, P], bf16)
            make_identity(nc, ident_b)

            # xT4[p, t, c, j] = x[t*128+j, c*128+p] as bf16
            xT4 = const.tile([P, SQT, C, P], bf16)
            # qT/kT zero-padded from DK=64 to 128 partitions
            qT = const.tile([P, S], bf16)
            kT = const.tile([P, S], bf16)
            vT = const.tile([DK, S], bf16)
            v_sb = const.tile([P, U * DK], bf16)
            recip = const.tile([P, SQT], f32)   # 1/Z per query tile
            att_all = const.tile([P, SQT, DK], f32)
            nc.gpsimd.memset(qT[DK:, :], 0.0)
            nc.gpsimd.memset(kT[DK:, :], 0.0)

            # ---------------- setup phase (own PSUM pools) ----------------
            with (
                tc.tile_pool(name="xstage", bufs=1) as xstage,
                tc.tile_pool(name="trps", bufs=2, space="PSUM") as trps,
                tc.tile_pool(name="qkvps", bufs=2, space="PSUM") as qkvps,
            ):
                # weights: [512,64] f32 -> [128, 3, c, 64] bf16
                wstage = xstage.tile([P, 3, C, DK], f32, tag="wstage")
                for i, w_d in enumerate((wq_d, wk_d, wv_d)):
                    nc.sync.dma_start(
                        wstage[:, i], w_d.rearrange("(c p) k -> p c k", p=P))
                w_sb3 = const.tile([P, 3, C, DK], bf16)
                nc.vector.tensor_copy(w_sb3[:], wstage[:])

                # x: SWDGE DMA with fused f32->bf16 cast, in 4 chunks,
                # each followed by a DMA xbar transpose into xT4.
                xb = xstage.tile([P, SQT, D], bf16, tag="xb")
                for q in range(4):
                    csl = slice(4 * q, 4 * (q + 1))
                    nc.gpsimd.dma_start(
                        xb[:, csl],
                        x_d.rearrange("(t p) d -> p t d", p=P)[:, csl])
                    nc.sync.dma_start_transpose(
                        xT4[:, csl], xb[:, csl].rearrange("p t d -> p (t d)"))

                # qT, kT, vT = W.T @ xT
                for n in range(NCH):
                    for w_i, dst in ((0, qT), (1, kT), (2, vT)):
                        mm = qkvps.tile([DK, 512], f32, tag="qkv")
                        for c in range(C):
                            nc.tensor.matmul(
                                mm[:], w_sb3[:, w_i, c, :],
                                xT4[:, 4 * n:4 * (n + 1), c, :],
                                start=(c == 0), stop=(c == C - 1))
                        nc.vector.tensor_copy(
                            dst[:DK, n * 512:(n + 1) * 512], mm[:])

                # v natural [128, u*64] via PE transpose of vT
                for g in range(2):
                    trp = trps.tile([P, 512], bf16, tag="tr")
                    for j in range(8):
                        u = g * 8 + j
                        nc.tensor.transpose(
                            trp[:, j * DK:(j + 1) * DK],
                            vT[:, u * P:(u + 1) * P], ident_b[:DK, :DK])
                    nc.vector.tensor_copy(
                        v_sb[:, g * 512:(g + 1) * 512], trp[:])

            # ---------------- main loop (own PSUM pools) ----------------
            with (
                tc.tile_pool(name="expp", bufs=2) as expp,
                tc.tile_pool(name="ptc", bufs=2) as ptc,
                tc.tile_pool(name="pout", bufs=3) as pout,
                tc.tile_pool(name="spsum", bufs=2, space="PSUM") as spsum,
                tc.tile_pool(name="trpsum", bufs=2, space="PSUM") as trpsum,
                tc.tile_pool(name="attpsum", bufs=2, space="PSUM") as attps,
            ):
                expS_hist = {}
                ptcols_hist = {}

                def stage_s(t):
                    # S_t matmuls + exp + normalize + weights DMA out
                    tsl = slice(t * P, (t + 1) * P)
                    expS = expp.tile([P, S], bf16, tag="exp")
                    zp = expp.tile([P, 2], f32, tag="zp")
                    for h in range(2):
                        sp = spsum.tile([P, 1024], f32, tag="s")
                        for i in range(2):
                            n = h * 2 + i
                            nc.tensor.matmul(
                                sp[:, i * 512:(i + 1) * 512],
                                qT[:, tsl], kT[:, n * 512:(n + 1) * 512],
                                start=True, stop=True)
                        nc.scalar.activation(
                            expS[:, h * 1024:(h + 1) * 1024], sp[:], Exp,
                            scale=0.125, accum_out=zp[:, h:h + 1])
                    zs = expp.tile([P, 1], f32, tag="zs")
                    nc.vector.tensor_add(zs[:], zp[:, 0:1], zp[:, 1:2])
                    nc.vector.reciprocal(recip[:, t:t + 1], zs[:])
                    p_t = pout.tile([P, S], bf16, tag="p")
                    nc.vector.tensor_scalar_mul(
                        p_t[:], expS[:], recip[:, t:t + 1])
                    nc.sync.dma_start(wts_d[tsl, :], p_t[:])
                    expS_hist[t] = expS

                def stage_tr(t):
                    # PE-transpose expS_t into ptcols_t [128, u*128]
                    expS = expS_hist.pop(t)
                    ptcols = ptc.tile([P, U * P], bf16, tag="ptc")
                    for g in range(4):
                        trp = trpsum.tile([P, 512], bf16, tag="tr")
                        for j in range(4):
                            u = g * 4 + j
                            nc.tensor.transpose(
                                trp[:, j * P:(j + 1) * P],
                                expS[:, u * P:(u + 1) * P], ident_b[:])
                        nc.vector.tensor_copy(
                            ptcols[:, g * 512:(g + 1) * 512].bitcast(i32),
                            trp[:].bitcast(i32))
                    ptcols_hist[t] = ptcols

                def stage_pv(t):
                    # att_t = sum_u PT_t[u].T @ v_u ; scale; DMA out
                    ptcols = ptcols_hist.pop(t)
                    ap = attps.tile([P, DK], f32, tag="att")
                    for u in range(U):
                        nc.tensor.matmul(
                            ap[:], ptcols[:, u * P:(u + 1) * P],
                            v_sb[:, u * DK:(u + 1) * DK],
                            start=(u == 0), stop=(u == U - 1))
                    nc.vector.tensor_scalar_mul(
                        att_all[:, t], ap[:], recip[:, t:t + 1])

                for t in range(SQT):
                    stage_s(t)
                    if t >= 1:
                        stage_tr(t - 1)
                    if t >= 2:
                        stage_pv(t - 2)
                stage_tr(SQT - 1)
                stage_pv(SQT - 2)
                stage_pv(SQT - 1)
                nc.sync.dma_start(
                    att_d.rearrange("(t p) k -> p t k", p=P), att_all[:])

    nc.compile()
    return nc


def get_nc():
    if "nc" not in _CACHE:
        _CACHE["nc"] = _build()
    return _CACHE["nc"]


def kernel(inputs, W_q, W_k, W_v):
    from concourse.bass_utils import run_bass_kernel_spmd

    nc = get_nc()
    inputs = np.ascontiguousarray(inputs, dtype=np.float32)
    in_maps = [
        {
            "x": inputs[b],
            "wq": np.ascontiguousarray(W_q, dtype=np.float32),
            "wk": np.ascontiguousarray(W_k, dtype=np.float32),
            "wv": np.ascontiguousarray(W_v, dtype=np.float32),
        }
        for b in range(B)
    ]
    res = run_bass_kernel_spmd(nc, in_maps, core_ids=list(range(B)))
    att = np.stack([res.results[b]["att"] for b in range(B)])
    wts = np.stack(
        [res.results[b]["wts"].astype(np.float32) for b in range(B)])
    return att, wts
